# revision 29
# baseline (speedup 1.0000x reference)
"""Trainium2 Bass kernel for GQA attention (nn_Attention_15015205667492).

Reference computation (per batch b, seq s=2048, d=2048):
  q = (x @ wq)  -> 32 heads x 64     (RoPE)
  k = (x @ wk)  ->  8 kv heads x 64  (RoPE)
  v = (x @ wv)  ->  8 kv heads x 64
  causal softmax(q k^T / 8) @ v  (GQA: kv head = q head // 4)
  out = attn @ wo

Sharding (8 cores): DP2 x TP4.
  core c: batch = c//4, head-group g = c%4 (Q heads 8g..8g+7, KV heads 2g, 2g+1).
  Each core computes attention for its 8 heads over its batch, writes the
  head-transposed attention output [512, S] bf16 to DRAM, AllGathers it within
  its 4-core batch group -> [2048, S], then computes a column slice of o_proj
  (wo[:, 512g:512(g+1)]) so per-core outputs are disjoint blocks of the final
  output (host-side unshard is pure concatenation).

Kernel layout choices:
  - x is passed transposed+bf16 (xT [d, s]) so all projections contract d on
    partitions.  Q/K come out transposed ([head-pair 128, s]) which is what
    the QK^T matmul wants as lhsT/rhs; V comes out natural ([s, kv 128]).
  - Scores are computed transposed: S^T[k, q] = kT.T @ qT per 128-k-block, so
    softmax numerator exp() runs on ScalarE and the AV matmul consumes P^T
    directly (no P transpose anywhere).
  - Softmax denominator comes free from the AV matmul: V is augmented with a
    ones column, so row 64 of the AV psum accumulates sum_k exp(s); the
    divide uses reciprocal_approx_fast (51 ULP, ample for softmax sums).
  - Causality is static: key blocks beyond the query block are skipped;
    diagonal blocks get a binary mask multiply on P^T, and far-diagonal
    blocks (j>=2) shrink the processed q-window to their visible range.
  - The whole pipeline is ordered for overlap: input DMAs are issued in
    consumption order (wk/wv/xT interleaved per d-chunk first) so the
    K projection starts ~2us in; K/V/Q projection and attention proceed per
    seq half; o_proj blocks are interleaved between attention blocks.
  - The softmax normalization chain has no PE instructions: VectorE drains
    the AV psum, GpSimd broadcasts the denominator row, VectorE does
    reciprocal+multiply, and GpSimd issues the collective-input DMA, so the
    PE stream of the next head pair is never blocked.
"""

import sys

sys.path.insert(0, "/opt/trn_rl_repo")

import numpy as np
import ml_dtypes

N_CORES = 8
H, KVH, HD = 32, 8, 64
RG = [[0, 1, 2, 3], [4, 5, 6, 7]]

_cache = {}


def build_program(S=2048, D=2048, enable_asserts=False, NO_CC=False, bench_iters=0):
    import concourse.mybir as mybir
    import concourse.tile as tile
    from concourse import bacc, bass_isa

    f32 = mybir.dt.float32
    bf16 = mybir.dt.bfloat16
    Exp = mybir.ActivationFunctionType.Exp

    DC = D // 128       # contraction chunks for projections (16)
    QB = S // 512       # query blocks (512 q rows each)
    KB = S // 128       # key blocks (16)
    DOUT = D // 4       # output column slice per core
    HC = (H * HD) // 128  # o_proj contraction chunks (16)
    HS = S // 2         # seq half

    nc = bacc.Bacc(
        "TRN2",
        target_bir_lowering=False,
        debug=False,
        enable_asserts=enable_asserts,
        num_devices=N_CORES,
    )

    xT_d = nc.dram_tensor("xT", [D, S], bf16, kind="ExternalInput")
    wq_d = nc.dram_tensor("wq", [D, 512], bf16, kind="ExternalInput")
    wk_d = nc.dram_tensor("wk", [D, 128], bf16, kind="ExternalInput")
    wv_d = nc.dram_tensor("wv", [D, 128], bf16, kind="ExternalInput")
    wo_d = nc.dram_tensor("wo", [H * HD, DOUT], bf16, kind="ExternalInput")
    cos_d = nc.dram_tensor("cos2", [128, S], bf16, kind="ExternalInput")
    sin_d = nc.dram_tensor("sinsw2", [128, S], bf16, kind="ExternalInput")
    rot_d = nc.dram_tensor("rot", [128, 128], bf16, kind="ExternalInput")
    msk_d = nc.dram_tensor("masks", [128, 2, 1024], bf16, kind="ExternalInput")
    out_d = nc.dram_tensor("out", [S, DOUT], f32, kind="ExternalOutput")

    with tile.TileContext(nc) as tc:
        with (
            tc.tile_pool(name="const", bufs=1) as const,
            tc.tile_pool(name="psA", bufs=2, space="PSUM") as psA,
            tc.tile_pool(name="psAV", bufs=1, space="PSUM") as psAV,
            tc.tile_pool(name="psP", bufs=2, space="PSUM") as psP,
            tc.tile_pool(name="work", bufs=2) as work,
            tc.tile_pool(name="dram", bufs=1, space="DRAM") as dram,
        ):
            # ---------------- SBUF tiles (persistent) ----------------
            xt = [const.tile([128, S], bf16, name=f"xt{i}", tag=f"xt{i}") for i in range(DC)]
            wq_t = [const.tile([128, 512], bf16, name=f"wq{i}", tag=f"wq{i}") for i in range(DC)]
            wk_t = [const.tile([128, 128], bf16, name=f"wk{i}", tag=f"wk{i}") for i in range(DC)]
            wv_t = [const.tile([128, 128], bf16, name=f"wv{i}", tag=f"wv{i}") for i in range(DC)]
            wo_t = [const.tile([128, DOUT], bf16, name=f"wo{i}", tag=f"wo{i}") for i in range(HC)]
            cos_sb = const.tile([128, S], bf16, name="cos", tag="cos")
            sin_sb = const.tile([128, S], bf16, name="sin", tag="sin")
            rot_sb = const.tile([128, 128], bf16, name="rot", tag="rot")
            msk_sb = const.tile([128, 2, 1024], bf16, name="msk", tag="msk")
            krope = const.tile([128, S], bf16, name="krope", tag="krope")
            ones_sb = const.tile([65, 64], f32, name="ones", tag="ones")
            kTd = [const.tile([128, S], bf16, name=f"kTd{h}", tag=f"kTd{h}") for h in range(2)]
            qT = [const.tile([128, S], bf16, name=f"qT{p}", tag=f"qT{p}") for p in range(4)]
            v_sb = [const.tile([128, 132], bf16, name=f"v{kb}", tag=f"v{kb}") for kb in range(KB)]

            # ---------------- input DMA issue, consumption order ----------
            # d-chunk interleave (wk, wv, xT-half0) so the K/V projections'
            # psum accumulation starts as soon as the first chunks land.
            for i in range(DC):
                nc.sync.dma_start(out=wk_t[i][:], in_=wk_d[128 * i : 128 * (i + 1), :])
                nc.sync.dma_start(out=wv_t[i][:], in_=wv_d[128 * i : 128 * (i + 1), :])
                nc.sync.dma_start(out=xt[i][:, 0:HS], in_=xT_d[128 * i : 128 * (i + 1), 0:HS])
            nc.sync.dma_start(out=rot_sb[:], in_=rot_d[:, :])
            nc.sync.dma_start(out=cos_sb[:, 0:HS], in_=cos_d[:, 0:HS])
            nc.sync.dma_start(out=sin_sb[:, 0:HS], in_=sin_d[:, 0:HS])
            nc.sync.dma_start(out=msk_sb[:], in_=msk_d[:, :, :])
            for i in range(DC):
                nc.sync.dma_start(out=wq_t[i][:], in_=wq_d[128 * i : 128 * (i + 1), :])
            for i in range(DC):
                nc.sync.dma_start(out=xt[i][:, HS:S], in_=xT_d[128 * i : 128 * (i + 1), HS:S])
            nc.sync.dma_start(out=cos_sb[:, HS:S], in_=cos_d[:, HS:S])
            nc.sync.dma_start(out=sin_sb[:, HS:S], in_=sin_d[:, HS:S])
            for i in range(HC):
                nc.sync.dma_start(out=wo_t[i][:], in_=wo_d[128 * i : 128 * (i + 1), :])

            # gathers are split per head-pair-half: the first half (pairs
            # 0,1) fires mid-attention and overlaps the remaining pairs; wo
            # rows are host-permuted to match the (half, rank, pair) order.
            cc_in = [
                [
                    dram.tile([256, 512], bf16, name=f"cin{qb}{h}", tag=f"cin{qb}{h}")
                    for h in range(2)
                ]
                for qb in range(QB)
            ]
            cc_out = [
                [
                    dram.tile([1024, 512], bf16, name=f"cout{qb}{h}", tag=f"cout{qb}{h}")
                    for h in range(2)
                ]
                for qb in range(QB)
            ]

            def emit_body():
                nc.vector.memset(ones_sb[:], 1.0)

                # ---------------- projection + RoPE (generators) ----------
                # Emitted stepwise so attention can interleave these PE
                # matmuls into its exp-wait bubbles.  All psum drains go to
                # VectorE; ScalarE runs only exp during attention.
                def proj_rope_steps(w_tiles, col0, dest, c0, c1):
                    CH = c1 - c0
                    raw = work.tile([128, CH], bf16, name="raw", tag="raw", bufs=2)
                    tmp = work.tile([128, CH], bf16, name="ropetmp", tag="ropetmp", bufs=2)
                    for q2 in range(CH // 512):
                        pq = psP.tile([128, 512], f32, name="pq", tag="pp")
                        for dc in range(DC):
                            nc.tensor.matmul(
                                pq[:],
                                w_tiles[dc][:, col0 : col0 + 128],
                                xt[dc][:, c0 + 512 * q2 : c0 + 512 * (q2 + 1)],
                                start=(dc == 0),
                                stop=(dc == DC - 1),
                            )
                            if dc % 2 == 1:
                                yield
                        nc.vector.tensor_copy(out=raw[:, 512 * q2 : 512 * (q2 + 1)], in_=pq[:])
                    for q2 in range(CH // 512):
                        pr = psP.tile([128, 512], f32, name="pr", tag="pp")
                        nc.tensor.matmul(
                            pr[:],
                            rot_sb[:],
                            raw[:, 512 * q2 : 512 * (q2 + 1)],
                            start=True,
                            stop=True,
                        )
                        nc.vector.tensor_mul(
                            tmp[:, 512 * q2 : 512 * (q2 + 1)],
                            pr[:],
                            sin_sb[:, c0 + 512 * q2 : c0 + 512 * (q2 + 1)],
                        )
                        yield
                    nc.vector.tensor_mul(raw[:], raw[:], cos_sb[:, c0:c1])
                    nc.vector.tensor_add(dest[:, c0:c1], raw[:], tmp[:])
                    yield

                def kproj_steps(h):
                    c0, c1 = HS * h, HS * (h + 1)
                    yield from proj_rope_steps(wk_t, 0, krope, c0, c1)
                    # duplicate each kv head across both 64-partition halves so
                    # the two QK matmuls of a head pair land on disjoint PE
                    # row groups.
                    for hh in range(2):
                        nc.scalar.dma_start(
                            out=kTd[hh][0:64, c0:c1], in_=krope[64 * hh : 64 * hh + 64, c0:c1]
                        )
                        nc.scalar.dma_start(
                            out=kTd[hh][64:128, c0:c1], in_=krope[64 * hh : 64 * hh + 64, c0:c1]
                        )
                    yield

                def vproj_steps(h):
                    for kb in range(8 * h, 8 * (h + 1)):
                        vt = v_sb[kb]
                        nc.vector.memset(vt[:, 64:65], 1.0)
                        nc.vector.memset(vt[:, 129:130], 1.0)
                        pv = psP.tile([128, 128], f32, name="pv", tag="pp")
                        for dc in range(DC):
                            nc.tensor.matmul(
                                pv[:],
                                xt[dc][:, 128 * kb : 128 * (kb + 1)],
                                wv_t[dc][:],
                                start=(dc == 0),
                                stop=(dc == DC - 1),
                            )
                            if dc % 4 == 3:
                                yield
                        nc.vector.tensor_copy(out=vt[:, 0:64], in_=pv[:, 0:64])
                        nc.vector.tensor_copy(out=vt[:, 65:129], in_=pv[:, 64:128])

                def qproj_steps(h):
                    for p in range(4):
                        yield from proj_rope_steps(wq_t, 128 * p, qT[p], HS * h, HS * (h + 1))

                def oproj_dma(qb, h):
                    cct = []
                    for hc in range(HC // 2):
                        t = work.tile(
                            [128, 512], bf16, name=f"cct{8 * h + hc}",
                            tag=f"cct{8 * h + hc}", bufs=2
                        )
                        nc.sync.dma_start(
                            out=t[:], in_=cc_out[qb][h][128 * hc : 128 * (hc + 1), :]
                        )
                        cct.append(t)
                    return cct

                def oproj_steps(qb, cct):
                    # per rb-pair: both A-chunk accumulations first (groups
                    # kept open on the two psP banks), B chunks after — the
                    # late-arriving B data is reached ~8us later in the PE
                    # stream, past the gather-copy window.
                    for rbp in range(2):
                        po = [
                            psP.tile([128, DOUT], f32, name=f"po{k}", tag="pp")
                            for k in range(2)
                        ]
                        for k in range(2):
                            rb = 2 * rbp + k
                            for hc in range(HC // 2):
                                nc.tensor.matmul(
                                    po[k][:],
                                    cct[hc][:, 128 * rb : 128 * (rb + 1)],
                                    wo_t[hc][:],
                                    start=(hc == 0),
                                    stop=False,
                                )
                                if hc % 2 == 1:
                                    yield
                        for k in range(2):
                            rb = 2 * rbp + k
                            for hc in range(HC // 2, HC):
                                nc.tensor.matmul(
                                    po[k][:],
                                    cct[hc][:, 128 * rb : 128 * (rb + 1)],
                                    wo_t[hc][:],
                                    start=False,
                                    stop=(hc == HC - 1),
                                )
                                if hc % 2 == 1:
                                    yield
                            ot = work.tile([128, DOUT], f32, name="ot", tag="ot", bufs=2)
                            nc.vector.tensor_copy(out=ot[:], in_=po[k][:])
                            nc.sync.dma_start(
                                out=out_d[
                                    512 * qb + 128 * rb : 512 * qb + 128 * (rb + 1), :
                                ],
                                in_=ot[:],
                            )
                    yield

                def oproj_tail(qb, cct):
                    # A-chunk accumulation for ALL row blocks first (these are
                    # ready before the final half-gather lands), keeping the 4
                    # psum groups open across banks; B chunks after.
                    po = []
                    for rb in range(4):
                        pool = psP if rb < 2 else psA
                        po.append(
                            pool.tile(
                                [128, DOUT], f32, name=f"pot{rb}",
                                tag=("pp" if rb < 2 else "ps"),
                            )
                        )
                    for rb in range(4):
                        for hc in range(HC // 2):
                            nc.tensor.matmul(
                                po[rb][:],
                                cct[hc][:, 128 * rb : 128 * (rb + 1)],
                                wo_t[hc][:],
                                start=(hc == 0),
                                stop=False,
                            )
                    for rb in range(4):
                        for hc in range(HC // 2, HC):
                            nc.tensor.matmul(
                                po[rb][:],
                                cct[hc][:, 128 * rb : 128 * (rb + 1)],
                                wo_t[hc][:],
                                start=False,
                                stop=(hc == HC - 1),
                            )
                        ot = work.tile([128, DOUT], f32, name="ot", tag="ot", bufs=2)
                        nc.vector.tensor_copy(out=ot[:], in_=po[rb][:])
                        nc.sync.dma_start(
                            out=out_d[512 * qb + 128 * rb : 512 * qb + 128 * (rb + 1), :],
                            in_=ot[:],
                        )

                # PE-bubble filler machinery
                filler = [None]

                def pull(n):
                    while n > 0 and filler[0] is not None:
                        try:
                            next(filler[0])
                            n -= 1
                        except StopIteration:
                            filler[0] = None

                def run_all(gen):
                    for _ in gen:
                        pass

                def chain_gens(*gens):
                    for g in gens:
                        yield from g

                # ---------------- attention ----------------
                # The denominator broadcast + divide + collective-input DMA
                # for pair i is emitted AFTER pair i+1's matmul chain, so the
                # PE-queue head never waits on the psum drain feeding it.
                deferred_norm = [None]

                def flush_norm():
                    if deferred_norm[0] is not None:
                        fn = deferred_norm[0]
                        deferred_norm[0] = None
                        fn()

                def attn_emit(qb, pull_from_pidx=0, on_ag_a=None, drain_filler=False):
                    kmax = 4 * (qb + 1)
                    for hg in range(2):  # kv head (local)
                        for p2 in range(2):  # head pair within kv group
                            pidx = 2 * hg + p2
                            pav = psAV.tile([65, 1024], f32, name="pav", tag="pav")
                            for kb in range(kmax):
                                # diagonal blocks only see queries q >= 128j:
                                # shrink the processed q-window to vw columns
                                j = kb - 4 * qb
                                vw = 512 - 128 * j if j >= 1 else 512
                                q0 = 512 * qb + (512 - vw)
                                ps = psA.tile([128, 1024], f32, name="ps", tag="ps")
                                for i in range(2):
                                    r0 = 64 * i
                                    nc.tensor.matmul(
                                        ps[:, 512 * i : 512 * i + vw],
                                        kTd[hg][r0 : r0 + 64, 128 * kb : 128 * (kb + 1)],
                                        qT[pidx][r0 : r0 + 64, q0 : q0 + vw],
                                        start=True,
                                        stop=True,
                                    )
                                pt = work.tile([128, 1024], bf16, name="pt", tag="pt", bufs=2)
                                if vw == 512:
                                    nc.scalar.activation(
                                        out=pt[:], in_=ps[:], func=Exp, scale=0.125
                                    )
                                    if j == 0:
                                        nc.vector.tensor_mul(pt[:], pt[:], msk_sb[:, 0, :])
                                else:
                                    for i in range(2):
                                        sl = slice(512 * i, 512 * i + vw)
                                        nc.scalar.activation(
                                            out=pt[:, sl], in_=ps[:, sl], func=Exp, scale=0.125
                                        )
                                        # restricted tri mask == prefix of mask_0
                                        nc.vector.tensor_mul(
                                            pt[:, sl], pt[:, sl], msk_sb[:, 0, 0:vw]
                                        )
                                if pidx >= pull_from_pidx:
                                    pull(2)
                                for i in range(2):
                                    nc.tensor.matmul(
                                        pav[:, 512 * i + 512 - vw : 512 * (i + 1)],
                                        v_sb[kb][:, 65 * hg : 65 * hg + 65],
                                        pt[:, 512 * i : 512 * i + vw],
                                        start=(kb == 0),
                                        stop=(kb == kmax - 1),
                                    )
                            # normalize: out = O^T_unnorm * (1/colsum), with
                            # the denominator row broadcast by a K=1 matmul
                            # against a ones row (deferred, see above).
                            flush_norm()
                            ou = work.tile([65, 1024], f32, name="ou", tag="ou", bufs=1)
                            nc.vector.tensor_copy(out=ou[:], in_=pav[:])

                            def norm_tail(ou=ou, pidx=pidx, qb=qb):
                                pb = psA.tile([64, 1024], f32, name="pb", tag="ps")
                                for i in range(2):
                                    nc.tensor.matmul(
                                        pb[:, 512 * i : 512 * (i + 1)],
                                        ones_sb[64:65, :],
                                        ou[64:65, 512 * i : 512 * (i + 1)],
                                        start=True,
                                        stop=True,
                                    )
                                rbc = work.tile(
                                    [64, 1024], f32, name="rbc", tag="rbc", bufs=1
                                )
                                nc.vector.reciprocal_approx_fast(out=rbc[:], in_=pb[:])
                                at = work.tile([64, 1024], bf16, name="at", tag="at", bufs=1)
                                nc.vector.tensor_mul(at[:], ou[0:64, :], rbc[:])
                                r0 = 128 * (pidx % 2)
                                for i in range(2):
                                    nc.scalar.dma_start(
                                        out=cc_in[qb][pidx // 2][
                                            r0 + 64 * i : r0 + 64 * (i + 1), :
                                        ],
                                        in_=at[:, 512 * i : 512 * (i + 1)],
                                    )

                            deferred_norm[0] = norm_tail
                            if pidx == 2:
                                emit_ag(qb, 0)
                                if on_ag_a is not None:
                                    on_ag_a()
                    flush_norm()
                    if drain_filler:
                        pull(10**9)
                    emit_ag(qb, 1)

                def emit_ag(qb, h):
                    if NO_CC:
                        nc.sync.dma_start(
                            out=cc_out[qb][h][0:256, :], in_=cc_in[qb][h][:, :]
                        )
                    else:
                        nc.gpsimd.collective_compute(
                            "AllGather",
                            mybir.AluOpType.bypass,
                            replica_groups=RG,
                            ins=[cc_in[qb][h].opt()],
                            outs=[cc_out[qb][h].opt()],
                        )

                # ---------------- schedule ----------------
                # o_proj gather-read DMAs are issued eagerly right after each
                # AllGather so the transfers queue immediately; the matmuls
                # are pulled later as attention bubble-filler.
                ccts = {}

                def hoist_ccta(qb):
                    return lambda: ccts.__setitem__(qb, oproj_dma(qb, 0))

                run_all(kproj_steps(0))
                run_all(vproj_steps(0))
                run_all(qproj_steps(0))
                filler[0] = chain_gens(kproj_steps(1), vproj_steps(1), qproj_steps(1))
                attn_emit(0)
                cct0 = oproj_dma(0, 0) + oproj_dma(0, 1)
                attn_emit(1)
                cct1 = oproj_dma(1, 0) + oproj_dma(1, 1)
                pull(10**9)
                filler[0] = oproj_steps(0, cct0)
                attn_emit(2)
                cct2 = oproj_dma(2, 0) + oproj_dma(2, 1)
                pull(10**9)
                filler[0] = chain_gens(oproj_steps(1, cct1), oproj_steps(2, cct2))
                attn_emit(3, pull_from_pidx=1)
                cct3 = oproj_dma(3, 0) + oproj_dma(3, 1)
                pull(10**9)
                oproj_tail(3, cct3)

            if bench_iters:
                with tc.For_i(0, bench_iters, 1, name="bench"):
                    emit_body()
            else:
                emit_body()

    nc.compile()
    return nc


def prep_inputs(x, cos, sin, wq, wk, wv, wo):
    """Shard + reformat full inputs into per-core input maps."""
    bf = ml_dtypes.bfloat16
    b, s, d = x.shape
    dout = d // 4
    cos2 = np.tile(np.ascontiguousarray(cos.T), (2, 1)).astype(bf)
    sinT = np.ascontiguousarray(sin.T)
    sinsw = np.concatenate([-sinT[:32], sinT[32:]], axis=0)
    sinsw2 = np.tile(sinsw, (2, 1)).astype(bf)
    # rotate-half permutation: tmp[i] = raw[sigma(i)]; out = R.T @ raw
    rotm = np.zeros((128, 128), np.float32)
    for i in range(128):
        j = (i // 64) * 64 + ((i % 64) + 32) % 64
        rotm[j, i] = 1.0
    rotm = rotm.astype(bf)
    k_loc = np.arange(128)[:, None]
    q_loc = np.arange(512)[None, :]
    ms = []
    for j in range(2):
        mj = (k_loc <= q_loc - 128 * j).astype(np.float32)
        ms.append(np.concatenate([mj, mj], axis=1))
    masks = np.stack(ms, axis=1).astype(bf)  # [128, 2, 1024]
    # per-pair-half gather: wo rows reordered to [half][rank][pair][i][hd]
    w4 = wo.reshape(4, 4, 2, 64, wo.shape[1])
    wo_perm = np.concatenate(
        [w4[:, 0:2].reshape(-1, wo.shape[1]), w4[:, 2:4].reshape(-1, wo.shape[1])]
    )

    in_maps = []
    for c in range(N_CORES):
        bb, g = divmod(c, 4)
        in_maps.append(
            {
                "xT": np.ascontiguousarray(x[bb].T).astype(bf),
                "wq": np.ascontiguousarray(wq[:, 512 * g : 512 * (g + 1)]).astype(bf),
                "wk": np.ascontiguousarray(wk[:, 128 * g : 128 * (g + 1)]).astype(bf),
                "wv": np.ascontiguousarray(wv[:, 128 * g : 128 * (g + 1)]).astype(bf),
                "wo": np.ascontiguousarray(wo_perm[:, dout * g : dout * (g + 1)]).astype(bf),
                "cos2": cos2,
                "sinsw2": sinsw2,
                "rot": rotm,
                "masks": masks,
            }
        )
    return in_maps


def assemble_output(results, b, s, d):
    full = np.empty((b, s, d), np.float32)
    dout = d // 4
    for c in range(N_CORES):
        bb, g = divmod(c, 4)
        full[bb][:, dout * g : dout * (g + 1)] = results[c]["out"]
    return full


def kernel(**inputs):
    x = np.asarray(inputs["x"], np.float32)
    b, s, d = x.shape
    key = (s, d)
    if key not in _cache:
        _cache[key] = build_program(S=s, D=d)
    nc = _cache[key]
    in_maps = prep_inputs(
        x,
        np.asarray(inputs["cos"], np.float32),
        np.asarray(inputs["sin"], np.float32),
        np.asarray(inputs["wq"], np.float32),
        np.asarray(inputs["wk"], np.float32),
        np.asarray(inputs["wv"], np.float32),
        np.asarray(inputs["wo"], np.float32),
    )
    from concourse.bass_utils import run_bass_kernel_spmd

    res = run_bass_kernel_spmd(nc, in_maps, core_ids=list(range(N_CORES)))
    return assemble_output(res.results, b, s, d)
